# revision 1
# baseline (speedup 1.0000x reference)
"""Trainium2 Bass kernel for the DAM train-batch loss (scatter_memory problem).

Strategy: shard the position axis n (1..511) across 8 cores (64 positions
each, core 7 padded with a dummy slot whose loss contribution is weighted
to zero).  Each core computes, for its positions n:

  A_n      = softmax over i<n of A_logits[n]          (H, N)   [exp + masked matmul]
  hat_n    = sequences @ A_n.T / rowsum               (B, H)   [via transposed matmuls]
  phi      = softmax(B_logits) @ memory.T             (H, M)   [replicated, tiny]
  score_n  = hat_n @ phi                              (B, M)
  den/num  = sum_m exp(score) {*, plus[m,n]}          (B,)     [ACT accum + DVE ttr]
  bce sum  = sum_b log(0.5 + targ*(num/den - 0.5))    partial scalar per b

The final mean over all (b, n) is assembled on the host from tiny per-core
partial sums (no cross-core collectives needed).
"""

import sys

sys.path.insert(0, "/opt/trn_rl_repo")

from contextlib import ExitStack

import ml_dtypes
import numpy as np

import concourse.bacc as bacc
import concourse.bass as bass
import concourse.tile as tile
from concourse import mybir
from concourse.bass_utils import run_bass_kernel_spmd
from concourse.masks import make_identity

F32 = mybir.dt.float32
F32R = mybir.dt.float32r
BF16 = mybir.dt.bfloat16
BF = ml_dtypes.bfloat16

N = 512          # sequence length
H = 64           # heads
M = 1024         # memories
B = 256          # batch
NL = 64          # positions per core
NPAIR = NL // 2  # position pairs per core
NCORES = 8

Exp = mybir.ActivationFunctionType.Exp
Ln = mybir.ActivationFunctionType.Ln
Copy = mybir.ActivationFunctionType.Copy
MULT = mybir.AluOpType.mult
ADD = mybir.AluOpType.add
SUB = mybir.AluOpType.subtract

_NC = None


def _build():
    global _NC
    if _NC is not None:
        return _NC

    nc = bacc.Bacc("TRN2", target_bir_lowering=False)

    a_sl = nc.dram_tensor("a_sl", [NL * H, N], F32, kind="ExternalInput")
    sqT = nc.dram_tensor("sqT", [N, 258], BF16, kind="ExternalInput")
    mkT = nc.dram_tensor("mkT", [N, NL], F32, kind="ExternalInput")
    memT = nc.dram_tensor("memT", [N, M], BF16, kind="ExternalInput")
    plusT = nc.dram_tensor("plusT", [NL, M], BF16, kind="ExternalInput")
    tg = nc.dram_tensor("tg", [B, NL], F32, kind="ExternalInput")
    cw = nc.dram_tensor("cw", [128, NL], F32, kind="ExternalInput")
    bl = nc.dram_tensor("bl", [H, N], F32, kind="ExternalInput")
    part_out = nc.dram_tensor("partial", [2, 128], F32, kind="ExternalOutput")

    with tile.TileContext(nc) as tc, ExitStack() as ctx:
        consts = ctx.enter_context(tc.tile_pool(name="consts", bufs=1))
        accs = ctx.enter_context(tc.tile_pool(name="accs", bufs=1))
        abuf = ctx.enter_context(tc.tile_pool(name="abuf", bufs=3))
        eab = ctx.enter_context(tc.tile_pool(name="eab", bufs=3))
        hatb = ctx.enter_context(tc.tile_pool(name="hatb", bufs=3))
        ebuf = ctx.enter_context(tc.tile_pool(name="ebuf", bufs=3))
        pbuf = ctx.enter_context(tc.tile_pool(name="pbuf", bufs=3))
        scr = ctx.enter_context(tc.tile_pool(name="scr", bufs=3))
        tpsum = ctx.enter_context(tc.tile_pool(name="tpsum", bufs=2, space="PSUM"))
        ntpsum = ctx.enter_context(tc.tile_pool(name="ntpsum", bufs=2, space="PSUM"))
        scpsum = ctx.enter_context(tc.tile_pool(name="scpsum", bufs=2, space="PSUM"))

        # ---- constants ----
        sq_sb = consts.tile([128, 4, 258], BF16)
        mk_sb = consts.tile([128, 4, NL], F32)
        mem_sb = consts.tile([128, 4, M], BF16)
        cw_sb = consts.tile([128, NL], F32)
        bl_sb = consts.tile([H, N], F32)
        for c in range(4):
            nc.sync.dma_start(sq_sb[:, c, :], sqT[c * 128:(c + 1) * 128, :])
            nc.sync.dma_start(mk_sb[:, c, :], mkT[c * 128:(c + 1) * 128, :])
            nc.sync.dma_start(mem_sb[:, c, :], memT[c * 128:(c + 1) * 128, :])
        nc.sync.dma_start(cw_sb[:], cw[:])
        nc.sync.dma_start(bl_sb[:], bl[:])
        ident = consts.tile([128, 128], BF16)
        make_identity(nc, ident)

        # ---- phi = softmax(B_logits) @ memory.T, shape (H, M), f32 ----
        ebx = consts.tile([H, N], BF16)
        sumB = consts.tile([H, 1], F32)
        nc.scalar.activation(ebx[:], bl_sb[:], Exp, accum_out=sumB[:])
        rB = consts.tile([H, 1], F32)
        nc.vector.reciprocal(rB[:], sumB[:])
        ebT_ps = tpsum.tile([128, 4, H], BF16, tag="tps")
        for k in range(4):
            nc.tensor.transpose(
                ebT_ps[:, k, :], ebx[:, k * 128:(k + 1) * 128], ident[0:H, 0:H]
            )
        ebT_sb = consts.tile([128, 4, H], BF16)
        for k in range(4):
            nc.vector.tensor_copy(ebT_sb[:, k, :], ebT_ps[:, k, :])
        phi_ps = scpsum.tile([128, M], F32, tag="scps")
        for mh in range(2):
            for k in range(4):
                nc.tensor.matmul(
                    phi_ps[0:H, mh * 512:(mh + 1) * 512],
                    lhsT=ebT_sb[:, k, :],
                    rhs=mem_sb[:, k, mh * 512:(mh + 1) * 512],
                    start=(k == 0),
                    stop=(k == 3),
                )
        phi_sb = consts.tile([H, M], F32R)
        for mh in range(2):
            nc.scalar.activation(
                phi_sb[:, mh * 512:(mh + 1) * 512],
                phi_ps[0:H, mh * 512:(mh + 1) * 512],
                Copy,
                scale=rB[:],
            )

        den_sb = accs.tile([128, 2, NL], F32)
        num_sb = accs.tile([128, 2, NL], F32)

        # ---- main loop over position pairs ----
        for t in range(NPAIR):
            L = abuf.tile([128, N], F32)
            nc.sync.dma_start(L[:], a_sl[t * 128:(t + 1) * 128, :])
            EA = eab.tile([128, N], BF16, tag="EA")
            nc.scalar.activation(EA[:], L[:], Exp)
            EAT_ps = tpsum.tile([128, 4, 128], BF16, tag="tps")
            for k in range(4):
                nc.tensor.transpose(
                    EAT_ps[:, k, :], EA[:, k * 128:(k + 1) * 128], ident[:]
                )
            EAm = eab.tile([128, 4, 2, H], BF16, tag="EAm")
            for k in range(4):
                for nh in range(2):
                    j = 2 * t + nh
                    nc.vector.tensor_scalar_mul(
                        EAm[:, k, nh, :],
                        EAT_ps[:, k, nh * H:(nh + 1) * H],
                        mk_sb[:, k, j:j + 1],
                    )
            nt_list = []
            for nh in range(2):
                nt_ps = ntpsum.tile([H, 258], F32, tag="nt")
                for k in range(4):
                    nc.tensor.matmul(
                        nt_ps[:],
                        lhsT=EAm[:, k, nh, :],
                        rhs=sq_sb[:, k, :],
                        start=(k == 0),
                        stop=(k == 3),
                    )
                nt_list.append(nt_ps)
            hat_list = []
            for nh in range(2):
                nt_ps = nt_list[nh]
                dinv = hatb.tile([H, 1], F32, tag=f"dinv{nh}")
                nc.vector.reciprocal(dinv[:], nt_ps[:, 256:257])
                hatT = hatb.tile([H, B], F32R, tag=f"hat{nh}")
                nc.scalar.activation(hatT[:], nt_ps[:, 0:B], Copy, scale=dinv[:])
                hat_list.append(hatT)

            for nh in range(2):
                j = 2 * t + nh
                hatT = hat_list[nh]
                pb = pbuf.tile([128, M], BF16)
                row = plusT[j:j + 1, :]
                src = bass.AP(
                    tensor=row.tensor, offset=row.offset,
                    ap=[[0, 128]] + [list(d) for d in row.ap[1:]],
                )
                nc.sync.dma_start(pb[:], src)
                for c in range(2):
                    sc_ps = scpsum.tile([128, M], F32, tag="scps")
                    for mh in range(2):
                        nc.tensor.matmul(
                            sc_ps[:, mh * 512:(mh + 1) * 512],
                            lhsT=hatT[:, c * 128:(c + 1) * 128],
                            rhs=phi_sb[:, mh * 512:(mh + 1) * 512],
                            start=True,
                            stop=True,
                        )
                    E_t = ebuf.tile([128, M], BF16)
                    nc.scalar.activation(
                        E_t[:], sc_ps[:], Exp,
                        accum_out=den_sb[:, c, j:j + 1],
                    )
                    sout = scr.tile([128, M], BF16)
                    nc.vector.scalar_tensor_tensor(
                        out=sout[:],
                        in0=E_t[:],
                        scalar=1.0,
                        in1=pb[:],
                        op0=MULT,
                        op1=MULT,
                        accum_out=num_sb[:, c, j:j + 1],
                    )

        # ---- tail: bce partials ----
        half_sb = accs.tile([128, 1], F32)
        nc.vector.memset(half_sb[:], 0.5)
        for c in range(2):
            tg_sb = accs.tile([128, NL], F32, tag=f"tg{c}")
            nc.sync.dma_start(tg_sb[:], tg[c * 128:(c + 1) * 128, :])
            rec = accs.tile([128, NL], F32, tag=f"rec{c}")
            nc.vector.reciprocal(rec[:], den_sb[:, c, :])
            pr = accs.tile([128, NL], F32, tag=f"pr{c}")
            nc.vector.tensor_mul(pr[:], num_sb[:, c, :], rec[:])
            nc.vector.tensor_scalar_max(pr[:], pr[:], 1e-6)
            nc.vector.tensor_scalar_min(pr[:], pr[:], 1.0 - 1e-6)
            qq = accs.tile([128, NL], F32, tag=f"qq{c}")
            nc.vector.scalar_tensor_tensor(
                out=qq[:], in0=pr[:], scalar=0.5, in1=tg_sb[:], op0=SUB, op1=MULT
            )
            lg = accs.tile([128, NL], F32, tag=f"lg{c}")
            nc.scalar.activation(lg[:], qq[:], Ln, bias=half_sb[:])
            ws = accs.tile([128, NL], F32, tag=f"ws{c}")
            rs = accs.tile([128, 1], F32, tag=f"rs{c}")
            nc.vector.scalar_tensor_tensor(
                out=ws[:], in0=lg[:], scalar=1.0, in1=cw_sb[:],
                op0=MULT, op1=MULT, accum_out=rs[:],
            )
            nc.sync.dma_start(part_out[c:c + 1, :], rs[:, 0:1])

    nc.compile()
    _NC = nc
    return nc


def _in_maps(sequences, memory, A_logits, B_logits):
    sequences = np.asarray(sequences, np.float32)
    memory = np.asarray(memory, np.float32)
    A_logits = np.asarray(A_logits, np.float32)
    B_logits = np.asarray(B_logits, np.float32)

    sqT_full = np.concatenate(
        [sequences.T, np.ones((N, 1), np.float32), np.zeros((N, 1), np.float32)],
        axis=1,
    ).astype(BF)  # (512, 258)
    memT_full = np.ascontiguousarray(memory.T).astype(BF)  # (512, 1024)

    maps = []
    for k in range(NCORES):
        n0 = 1 + NL * k
        n_real = np.arange(n0, n0 + NL)          # may include 512 (pad slot)
        ns = np.minimum(n_real, N - 1)           # clamped for data indexing
        a_sl = np.ascontiguousarray(
            A_logits[ns].reshape(NL * H, N)
        ).astype(np.float32)
        mk = (np.arange(N)[:, None] < n_real[None, :]).astype(np.float32)  # (512, 64)
        pl = np.ascontiguousarray((memory[:, ns].T > 0)).astype(BF)  # (64, 1024)
        t_raw = sequences[:, ns].copy()          # (256, 64)
        w = np.ones((128, NL), np.float32)
        pad = n_real > (N - 1)
        t_raw[:, pad] = 0.0
        w[:, pad] = 0.0
        maps.append({
            "a_sl": a_sl,
            "sqT": sqT_full,
            "mkT": mk,
            "memT": memT_full,
            "plusT": pl,
            "tg": np.ascontiguousarray(t_raw, dtype=np.float32),
            "cw": w,
            "bl": B_logits,
        })
    return maps


def _run(maps, trace=False):
    nc = _build()
    return run_bass_kernel_spmd(nc, maps, list(range(NCORES)), trace=trace)


def kernel(sequences, memory, A_logits, B_logits, _trace=False):
    maps = _in_maps(sequences, memory, A_logits, B_logits)
    res = _run(maps, trace=_trace)
    tot = 0.0
    for r in res.results:
        tot += r["partial"].astype(np.float64).sum()
    out = np.float32(-tot / (B * (N - 1)))
    if _trace:
        return out, res
    return out

